# revision 32
# baseline (speedup 1.0000x reference)
"""Trainium2 Bass kernel for nn_CNN_88098369175791.

Tiny attention/CNN hybrid (batch=1): two time-delay MHAs (E=119) over
sliding wav windows, argmax channel select, LayerNorm, four cross-modal
MHAs (E=16), and an MLP head. The whole model fits on one NeuronCore;
per the sharding hint the program is replicated on all 8 cores (pure
data parallel; with one sample every core computes the same result) and
core 0's output is returned.

Host-side prep does layout only (weight transposes, sliding-window
gathers, bias packing, ones-row augmentation so per-partition biases
ride along inside the matmuls); all arithmetic runs on device with
bf16 PE operands and fp32 PSUM accumulation.

Numerics notes:
- softmax skips the max-subtraction: logits here are provably tiny
  (|l| < 1.5), so exp() is safe and the exp can stream straight out of
  the logits matmul without waiting for a reduction;
- softmax normalization is deferred past the value matmuls and divided
  out where the normalizer lands on a partition axis;
- sigmoids are computed as 1/(1+exp(-z)) so ACT only ever loads the
  Sqrt and Exp tables (a table switch costs ~1.3us).
"""
import itertools
import os
import sys

for _p in ('/opt/trn_rl_repo', '/root/.axon_site/_ro/trn_rl_repo'):
    if os.path.isdir(_p) and _p not in sys.path:
        sys.path.insert(0, _p)

import numpy as np
from contextlib import ExitStack

import concourse.bass as bass
import concourse.tile as tile
from concourse import mybir
from concourse.bass_utils import run_bass_kernel_spmd
from bass_rust import add_dep_helper

F32 = mybir.dt.float32
AX = mybir.AxisListType.X
ALU = mybir.AluOpType
ACTF = mybir.ActivationFunctionType

WL = 140      # window length
TD = 14       # time-delay windows
OFC = 119     # positions / td embed dim
E2 = 16       # cross-modal embed dim
S_TD = float(OFC) ** -0.5
S_CM = float(E2) ** -0.5
N_CORES = 8

PE_MODE = os.environ.get('KPE', 'bf16')
PE_DT = mybir.dt.bfloat16 if PE_MODE == 'bf16' else mybir.dt.float32
PE_NP = np.float32
if PE_MODE == 'bf16':
    import ml_dtypes
    PE_NP = ml_dtypes.bfloat16

INPUT_NAMES = [
    "x", "td_in_w", "td_in_b", "td_out_w", "td_out_b",
    "cm_in_w", "cm_in_b", "cm_out_w", "cm_out_b",
    "mc_w", "mc_b", "max_fc_w", "max_fc_b", "proj_w",
    "ln_g", "ln_b", "fc_w", "fc_b", "out1_w", "out1_b", "out2_w", "out2_b",
]

# ---------------------------------------------------------------------------
# pack layouts (static: computed from shapes only)
# ---------------------------------------------------------------------------


def _mk_layout(specs):
    off = {}
    c = 0
    for name, p, f in specs:
        off[name] = (p, c, f)
        c += f
    return off, c


# PE-operand pack (dtype PE_DT). Order = DMA arrival order; chunk boundaries
# below keep the td-attention front of the kernel fed by the first chunk.
WPK_SPECS = [
    ('winA_aug', 120, TD),        # [wavA windows embed-major ; ones row]
    ('winB_aug', 120, TD),        # adjacent: winAB = joint [120, 28] slice
    ('winGap', 120, 46),          # A @cols 0:14, B @cols 32:46 (vp stacking)
    ('Qpe_aug', 120, 16),         # [eeg_q.T ; ones row]
    ('wqT_aug', 120, OFC),        # [Wq.T ; bq row]
    ('wkT_aug', 120, OFC),        # [Wk.T ; bk row]
    ('wvT_aug', 120, OFC),        # [Wv.T ; bv row]
    # ---- chunk 1 ends
    ('ident', 128, 128),
    ('woB', OFC, 120),            # [Wo | bo col]
    ('eegcm', 16, OFC),
    # ---- chunk 2 ends
    ('winT', TD, 2 * OFC),        # token-major windows [A | B]
    ('mcw0', 16, 1),
    ('mcw1', 16, 1),
    ('mfwT65', 65, 16),           # rows 0:16 = mfwA.T, 32:48 = mfwB.T, 64 = mfb
    ('projcat', 1, 32),
    ('ones16', 16, 1),
    ('stkE', 17, 112),            # [wk2T0 |. wq2T1 |. wq2T2] blocks @0/32/64
    ('stkE2', 17, 16),            # wk2T3 @0
    ('stkA', 17, 48),             # [wq2T0 |. wk2T1] blocks @0/32
    ('stkB', 17, 112),            # [wq2T3 |. .. wk2T2] blocks @0/64
    ('vstkE', 17, 32),            # [wv2T_aug0 | wv2T_aug3]
    ('vstk1', 17, 16),            # wv2T_aug1
    ('vstk2', 17, 16),            # wv2T_aug2
    ('wo2T0', 17, 16), ('wo2T1', 17, 16),
    ('wo2T2', 17, 16), ('wo2T3', 17, 16),
    # ---- chunk 3 ends
    ('o1aT', OFC, OFC),
    ('o1bT', OFC, OFC),
    ('o2T', OFC, 2),
]
WPK_OFF, WPK_F = _mk_layout(WPK_SPECS)
WPK_CHUNK_ENDS = ['wvT_aug', 'eegcm', 'wo2T3', 'o2T']

# f32 side pack: bias columns, DVE scalars, LN input
SPK_SPECS = [
    ('Qf32', OFC, 16),                         # first: tiny DMA, gates LN
    ('no1b', OFC, 1), ('no2b', 2, 1),          # negated (sigmoid-via-exp)
    ('mcb0', 16, 1), ('mcb1', 16, 1),
    ('lng', 16, 1), ('lnb', 16, 1),
    ('nfcw0', OFC, 1), ('nfcw1', OFC, 1),
    ('nfcb0', OFC, 1), ('nfcb1', OFC, 1),
    ('iota16', 1, 16), ('iota14', 1, TD),
]
SPK_OFF, SPK_F = _mk_layout(SPK_SPECS)


def _pack_arrays(inputs):
    """Host-side layout: gathers/transposes/padding only."""
    g = {k: np.asarray(inputs[k], dtype=np.float32) for k in INPUT_NAMES}
    x = g['x'][0, 0]                       # [18,140]
    wavA, eeg, wavB = x[0], x[1:17], x[17]
    eeg_q = eeg[:, WL - OFC:]              # [16,119]
    idx = np.arange(OFC)[:, None] + np.arange(TD)[None, :]
    wA_win = wavA[idx]                     # [119,14]
    wB_win = wavB[idx]

    def aug(m, extra_row):
        return np.concatenate([m, np.asarray(extra_row)[None, :]], axis=0)

    tdw, tdb = g['td_in_w'], g['td_in_b']
    w = {}
    w['winA_aug'] = aug(wA_win, np.ones(TD, np.float32))
    w['winB_aug'] = aug(wB_win, np.ones(TD, np.float32))
    winGap = np.zeros((120, 46), np.float32)
    winGap[:, 0:TD] = w['winA_aug']
    winGap[:, 32:32 + TD] = w['winB_aug']
    w['winGap'] = winGap
    w['Qpe_aug'] = aug(eeg_q.T, np.ones(16, np.float32))
    w['wqT_aug'] = aug(tdw[0:OFC].T, tdb[0:OFC])
    w['wkT_aug'] = aug(tdw[OFC:2 * OFC].T, tdb[OFC:2 * OFC])
    w['wvT_aug'] = aug(tdw[2 * OFC:].T, tdb[2 * OFC:])
    w['ident'] = np.eye(128, dtype=np.float32)
    w['woB'] = np.concatenate([g['td_out_w'], g['td_out_b'][:, None]], axis=1)
    w['eegcm'] = eeg_q
    w['winT'] = np.concatenate([wA_win.T, wB_win.T], axis=1)   # [14,238]
    w['mcw0'] = g['mc_w'][0][:, None]
    w['mcw1'] = g['mc_w'][1][:, None]
    mfwT65 = np.zeros((65, 16), np.float32)
    mfwT65[0:16] = g['max_fc_w'][:, 0:16].T
    mfwT65[32:48] = g['max_fc_w'][:, 16:32].T
    mfwT65[64] = g['max_fc_b']
    w['mfwT65'] = mfwT65
    w['projcat'] = g['proj_w'].reshape(1, 32)
    w['ones16'] = np.ones((16, 1), np.float32)

    cw, cb = g['cm_in_w'], g['cm_in_b']

    def qT(i):   # [17,16] = [Wq2_i.T ; bq2_i]
        return aug(cw[i][0:16].T, cb[i][0:16])

    def kT(i):
        return aug(cw[i][16:32].T, cb[i][16:32])

    def vT(i):
        return aug(cw[i][32:48].T, cb[i][32:48])

    stkE = np.zeros((17, 112), np.float32)
    stkE[:, 0:16] = kT(0)
    stkE[:, 32:48] = qT(1)
    stkE[:, 64:80] = qT(2)
    w['stkE'] = stkE
    w['stkE2'] = kT(3)
    stkA = np.zeros((17, 48), np.float32)
    stkA[:, 0:16] = qT(0)
    stkA[:, 32:48] = kT(1)
    w['stkA'] = stkA
    stkB = np.zeros((17, 112), np.float32)
    stkB[:, 0:16] = qT(3)
    stkB[:, 64:80] = kT(2)
    w['stkB'] = stkB
    w['vstkE'] = np.concatenate([vT(0), vT(3)], axis=1)
    w['vstk1'] = vT(1)
    w['vstk2'] = vT(2)
    for i in range(4):
        w[f'wo2T{i}'] = aug(g['cm_out_w'][i].T, g['cm_out_b'][i])
    w['o1aT'] = g['out1_w'][:, 0:OFC].T
    w['o1bT'] = g['out1_w'][:, OFC:].T
    w['o2T'] = g['out2_w'].T

    wpk = np.zeros((128, WPK_F), dtype=PE_NP)
    for name, (p, c0, f) in WPK_OFF.items():
        wpk[0:p, c0:c0 + f] = w[name].astype(PE_NP)

    s = {}
    s['no1b'] = -g['out1_b'][:, None]
    s['no2b'] = -g['out2_b'][:, None]
    s['mcb0'] = np.full((16, 1), g['mc_b'][0], np.float32)
    s['mcb1'] = np.full((16, 1), g['mc_b'][1], np.float32)
    s['lng'] = g['ln_g'][:, None]
    s['lnb'] = g['ln_b'][:, None]
    s['nfcw0'] = np.full((OFC, 1), -g['fc_w'][0], np.float32)
    s['nfcw1'] = np.full((OFC, 1), -g['fc_w'][1], np.float32)
    s['nfcb0'] = np.full((OFC, 1), -g['fc_b'][0], np.float32)
    s['nfcb1'] = np.full((OFC, 1), -g['fc_b'][1], np.float32)
    s['iota16'] = (np.arange(16, dtype=np.float32) / 1024.0)[None, :]
    s['iota14'] = (np.arange(TD, dtype=np.float32) / 1024.0)[None, :]
    s['Qf32'] = eeg_q.T

    spk = np.zeros((128, SPK_F), dtype=np.float32)
    for name, (p, c0, f) in SPK_OFF.items():
        spk[0:p, c0:c0 + f] = s[name]
    return wpk, spk


# ---------------------------------------------------------------------------
# BIR post-processing: the container's walrus encodes at most one sem-wait
# per instruction; hoist excess waits onto injected NoOp carriers.
# ---------------------------------------------------------------------------


def _split_sync_waits(nc, maxw=1):
    n_new = 0
    for f in nc.m.functions:
        for bb in f.blocks:
            new_insts = []
            for inst in bb.instructions:
                si = inst.sync_info
                if si is not None and si.on_wait and len(si.on_wait) > maxw:
                    waits = list(si.on_wait)
                    keep, extra = waits[:maxw], waits[maxw:]
                    while extra:
                        chunk, extra = extra[:maxw], extra[maxw:]
                        carrier = mybir.InstNoOp(
                            name=f"I-waitsplit-{n_new}",
                            engine=inst.engine,
                            ins=[],
                            outs=[],
                            sync_info=mybir.SyncInfo(on_wait=chunk,
                                                     on_update=[]),
                        )
                        n_new += 1
                        new_insts.append(carrier)
                    si.on_wait = keep
                new_insts.append(inst)
            bb.instructions[:] = new_insts
    return n_new


# ---------------------------------------------------------------------------
# device program
# ---------------------------------------------------------------------------


def _slim_tail(nc):
    """Drop the post-reset all-engine barrier at the end of the tile
    block: every engine halts right after it, the runtime only signals
    completion once all engines halt, and the sem reset it guards has
    already happened under barrier #1."""
    for f in nc.m.functions:
        for bb in f.blocks:
            if not bb.name.endswith('_end'):
                continue
            idx = None
            for i, inst in enumerate(bb.instructions):
                if isinstance(inst, mybir.InstDrain) and getattr(
                        inst, 'is_reset_sema', False):
                    idx = i
            if idx is None:
                continue
            # keep through the reset drain + its ISA payload; drop the
            # trailing barrier (Drain/EventSemaphore pairs)
            keep = bb.instructions[:idx + 1]
            for inst in bb.instructions[idx + 1:]:
                if isinstance(inst, (mybir.InstDrain,
                                     mybir.InstEventSemaphore)):
                    continue
                keep.append(inst)
            bb.instructions[:] = keep


def _body(tc, wpk_t, spk_t, y_ap, ctx):
    nc = tc.nc
    sb = ctx.enter_context(tc.tile_pool(name='sb', bufs=1))
    pp = ctx.enter_context(tc.tile_pool(name='ps', bufs=8, space='PSUM'))
    cnt = itertools.count()

    wpk = sb.tile([128, WPK_F], PE_DT, tag='wpk', name='wpk')
    spk = sb.tile([128, SPK_F], F32, tag='spk', name='spk')
    wap = wpk_t.ap()
    qf_cols = SPK_OFF['Qf32'][2]
    nc.gpsimd.dma_start(spk[:, 0:qf_cols], spk_t.ap()[:, 0:qf_cols])
    c0 = 0
    for k, endname in enumerate(WPK_CHUNK_ENDS):
        p_, cb_, f_ = WPK_OFF[endname]
        c1 = cb_ + f_
        nc.sync.dma_start(wpk[:, c0:c1], wap[:, c0:c1])
        if k == 0:
            nc.sync.dma_start(spk[:, qf_cols:], spk_t.ap()[:, qf_cols:])
        c0 = c1

    def W(name):
        p, c0, f = WPK_OFF[name]
        return wpk[0:p, c0:c0 + f]

    def Wj(name_a, name_b, p):
        pa, ca, fa = WPK_OFF[name_a]
        pb, cb, fb = WPK_OFF[name_b]
        assert ca + fa == cb
        return wpk[0:p, ca:cb + fb]

    def C(name):
        p, c0, f = SPK_OFF[name]
        return spk[0:p, c0:c0 + f]

    def S(p, f, dt=None):
        n = next(cnt)
        return sb.tile([p, f], dt or PE_DT, tag=f's{n}', name=f's{n}')

    def P(p, f, dt=F32):
        return pp.tile([p, f], dt, tag='ps', name=f'ps{next(cnt)}')

    def mm(m, n, lhsT, rhs):
        o = P(m, n)
        nc.tensor.matmul(o[:, :], lhsT, rhs, start=True, stop=True)
        return o

    def to_sb(psum, p, f, dt=None):
        t = S(p, f, dt)
        nc.vector.tensor_copy(t[:, :], psum[:, :])
        return t

    ident = W('ident')

    def peT(in_ap, p, f):
        o = P(f, p, PE_DT)
        nc.tensor.transpose(o[:, :], in_ap, ident[0:p, 0:p])
        return o

    def peT_sb(in_ap, p, f, dt=None):
        return to_sb(peT(in_ap, p, f), f, p, dt)

    INT32 = mybir.dt.int32

    # ---- LayerNorm stats on DVE during the DMA window; the Sqrt is ACT's
    #      first instruction (its table preloads for free), the Exp table
    #      load follows right behind it via a dummy op ----
    Qf = C('Qf32')                                   # [119,16] f32
    ssum = S(OFC, 1, F32)
    nc.vector.reduce_sum(ssum[:, :], Qf, axis=AX)
    sq = S(OFC, 16, F32)
    nc.vector.tensor_mul(sq[:, :], Qf, Qf)
    s2 = S(OFC, 1, F32)
    nc.vector.reduce_sum(s2[:, :], sq[:, :], axis=AX)
    nc.vector.tensor_scalar_mul(s2[:, :], s2[:, :], 1.0 / 16.0)
    mu = S(OFC, 1, F32)
    nc.vector.tensor_scalar_mul(mu[:, :], ssum[:, :], 1.0 / 16.0)
    musq = S(OFC, 1, F32)
    nc.vector.tensor_mul(musq[:, :], mu[:, :], mu[:, :])
    var = S(OFC, 1, F32)
    nc.vector.tensor_sub(var[:, :], s2[:, :], musq[:, :])
    nc.vector.tensor_scalar_add(var[:, :], var[:, :], 1e-5)
    xc = S(OFC, 16, F32)
    nc.vector.tensor_scalar_sub(xc[:, :], Qf, mu[:, 0:1])
    dume = S(1, 1, F32)
    nc.scalar.activation(dume[:, :], nc.const_aps.tensor(0.0, (1, 1)),
                         ACTF.Exp)

    # ---- time-delay attention front (PE) ----
    Qaug = W('Qpe_aug')                              # [120,16]
    winAB = Wj('winA_aug', 'winB_aug', 120)          # [120,28]
    QP = to_sb(mm(OFC, 16, W('wqT_aug'), Qaug), OFC, 16)
    KP = to_sb(mm(OFC, 2 * TD, W('wkT_aug'), winAB), OFC, 2 * TD)
    LG = mm(16, 2 * TD, QP[:, :], KP[:, :])          # [16,28]
    vpG = to_sb(mm(46, OFC, W('winGap'), W('wvT_aug')), 46, OFC)
    u0 = to_sb(mm(OFC, 1, W('eegcm'), W('mcw0')), OFC, 1)
    u1 = to_sb(mm(OFC, 1, W('eegcm'), W('mcw1')), OFC, 1)
    z0 = to_sb(mm(120, 1, W('woB'), u0[:, :]), 120, 1)
    z1 = to_sb(mm(120, 1, W('woB'), u1[:, :]), 120, 1)

    # ---- td softmax (no max-subtraction; normalization deferred) ----
    attn = S(16, 46)                                 # A @0:14, B @32:46
    nc.gpsimd.memset(attn[:, :], 0.0)
    nc.scalar.activation(attn[:, 0:TD], LG[:, 0:TD], ACTF.Exp, scale=S_TD)
    tdexpB = nc.scalar.activation(attn[:, 32:32 + TD], LG[:, TD:2 * TD],
                                  ACTF.Exp, scale=S_TD)
    # LN sqrt AFTER the td exps (its table switch rides the slack before
    # eegln is needed), then a dummy exp to switch the table right back
    # before the cm softmaxes
    std = S(OFC, 1, F32)
    sq_i = nc.scalar.activation(std[:, :], var[:, :], ACTF.Sqrt)
    dume2 = S(1, 1, F32)
    de_i = nc.scalar.activation(dume2[:, :], nc.const_aps.tensor(0.0, (1, 1)),
                                ACTF.Exp)
    add_dep_helper(sq_i.ins, tdexpB.ins, sync=False,
                   reason='ACT table order')
    add_dep_helper(de_i.ins, sq_i.ins, sync=False, reason='ACT table order')
    sums, rsums = [], []
    for h in range(2):
        sm = S(16, 1, F32)
        nc.vector.reduce_sum(sm[:, :], attn[:, 32 * h:32 * h + TD], axis=AX)
        rs = S(16, 1, F32)
        nc.vector.reciprocal(rs[:, :], sm[:, :])
        sums.append(sm)
        rsums.append(rs)
    attnT = peT_sb(attn[:, :], 16, 46)               # [46,16]
    OPs = S(128, 32)                                 # row 119 stays 1.0
    nc.gpsimd.memset(OPs[:, :], 1.0)
    opP = P(OFC, 32)
    nc.tensor.matmul(opP[:, 0:16], vpG[0:TD, :], attnT[0:TD, :],
                     start=True, stop=True)
    nc.tensor.matmul(opP[:, 16:32], vpG[32:32 + TD, :],
                     attnT[32:32 + TD, :], start=True, stop=True)
    nc.vector.tensor_copy(OPs[0:OFC, :], opP[:, :])

    # ---- select_max: vX = OPs_aug.T @ (Wo_aug @ uX) — the z vectors are
    #      precomputed off the critical chain, so the scores follow the
    #      OPs copy directly ----
    cat65 = S(65, 1)
    nc.gpsimd.memset(cat65[:, :], 1.0)
    vAp = mm(16, 1, OPs[0:120, 0:16], z0[:, :])
    vAn = S(16, 1, F32)
    nc.vector.tensor_mul(vAn[:, :], vAp[:, :], rsums[0][:, :])
    nc.vector.tensor_scalar(cat65[0:16, 0:1], vAn[:, :], C('mcb0'), 0.0,
                            op0=ALU.add, op1=ALU.max)
    vBp = mm(16, 1, OPs[0:120, 16:32], z1[:, :])
    vBn = S(16, 1, F32)
    nc.vector.tensor_mul(vBn[:, :], vBp[:, :], rsums[1][:, :])
    nc.vector.tensor_scalar(cat65[32:48, 0:1], vBn[:, :], C('mcb1'), 0.0,
                            op0=ALU.add, op1=ALU.max)
    wtp = mm(1, 16, cat65[:, :], W('mfwT65'))        # [1,16] incl. bias row
    mxw = S(1, 1, F32)
    nc.vector.reduce_max(mxw[:, :], wtp[:, :], axis=AX)
    eq = S(1, 16, F32)
    nc.vector.tensor_scalar(eq[:, :], wtp[:, :], mxw[0:1, 0:1], None,
                            op0=ALU.is_equal)
    msk = S(1, 16, F32)
    nc.vector.tensor_sub(msk[:, :], C('iota16'), eq[:, :])
    mi = S(1, 1, F32)
    nc.vector.tensor_reduce(mi[:, :], msk[:, :], axis=AX, op=ALU.min)
    mic = S(1, 1, F32)
    nc.vector.tensor_scalar(mic[:, :], mi[:, :], 1.0, float(TD - 1) / 1024.0,
                            op0=ALU.add, op1=ALU.min)
    ohr = S(1, TD)
    nc.vector.tensor_scalar(ohr[:, :], C('iota14'), mic[0:1, 0:1], None,
                            op0=ALU.is_equal)
    # (oh.T @ projcat) gives [14,32] = [oh*proj0 | oh*proj1]; multiplying
    # by the token-major windows selects row mi and applies the projection
    # in one step: wX_p = (oh*projX).T @ winT_X
    M1 = to_sb(mm(TD, 32, ohr[:, :], W('projcat')), TD, 32)
    Pp = P(16, 2 * OFC)
    nc.tensor.matmul(Pp[:, 0:OFC], M1[:, 0:16], W('winT')[:, 0:OFC],
                     start=True, stop=True)
    nc.tensor.matmul(Pp[:, OFC:2 * OFC], M1[:, 16:32],
                     W('winT')[:, OFC:2 * OFC], start=True, stop=True)
    PAB = S(32, 2 * OFC)                             # row 16 stays 1.0
    nc.gpsimd.memset(PAB[:, :], 1.0)
    nc.vector.tensor_copy(PAB[0:16, :], Pp[:, :])

    # ---- LayerNorm tail (Newton rsqrt on DVE) + eln-side cm projections;
    #      emitted after the select chain so its DVE/PE work fills gaps
    #      without delaying the critical path (eegln isn't needed until the
    #      cm logits) ----
    rstd = S(OFC, 1, F32)
    nc.vector.reciprocal(rstd[:, :], std[:, :])
    xn = S(OFC, 16)
    nc.vector.tensor_scalar_mul(xn[:, :], xc[:, :], rstd[:, 0:1])
    LNp = peT(xn[:, :], OFC, 16)                     # psum [16,119]
    eegln = S(32, OFC)                               # row 16 stays 1.0
    nc.gpsimd.memset(eegln[:, :], 1.0)
    nc.vector.tensor_scalar(eegln[0:16, :], LNp[:, :], C('lng'), C('lnb'),
                            op0=ALU.mult, op1=ALU.add)
    eln17 = eegln[0:17, :]
    QKe = to_sb(mm(112, OFC, W('stkE'), eln17), 112, OFC)
    KP2_3 = to_sb(mm(16, OFC, W('stkE2'), eln17), 16, OFC)
    vpE = to_sb(mm(OFC, 32, eln17, W('vstkE')), OFC, 32)   # [119, v0|v3]

    # ---- cross-modal attention, 4 heads, stage-major ----
    wA17 = PAB[0:17, 0:OFC]
    wB17 = PAB[0:17, OFC:2 * OFC]
    QKa = to_sb(mm(48, OFC, W('stkA'), wA17), 48, OFC)
    QKb = to_sb(mm(112, OFC, W('stkB'), wB17), 112, OFC)
    vp1 = to_sb(mm(OFC, 16, wA17, W('vstk1')), OFC, 16)
    vp2_ = to_sb(mm(OFC, 16, wB17, W('vstk2')), OFC, 16)
    qp2 = [QKa[0:16, :], QKe[32:48, :], QKe[64:80, :], QKb[0:16, :]]
    kp2 = [QKe[0:16, :], QKa[32:48, :], QKb[64:80, :], KP2_3[:, :]]
    vp2 = [vpE[:, 0:16], vp1[:, :], vp2_[:, :], vpE[:, 16:32]]
    LG2s = [mm(OFC, OFC, qp2[i], kp2[i]) for i in range(4)]
    ex2s, sm2s = [], []
    for i in range(4):
        ex2 = S(OFC, OFC)
        nc.scalar.activation(ex2[:, :], LG2s[i][:, :], ACTF.Exp, scale=S_CM)
        ex2s.append(ex2)
        sm2 = S(OFC, 1, F32)
        nc.vector.reduce_sum(sm2[:, :], ex2[:, :], axis=AX)
        sm2s.append(sm2)
    # normalizer products/reciprocals early (DVE order) so the head tail
    # only waits on the d0p/d1p matmuls
    nf0 = S(OFC, 1, F32)
    nc.vector.tensor_mul(nf0[:, :], sm2s[0][:, :], sm2s[1][:, :])
    nf1 = S(OFC, 1, F32)
    nc.vector.tensor_mul(nf1[:, :], sm2s[3][:, :], sm2s[2][:, :])
    rf0 = S(OFC, 1, F32)
    nc.vector.reciprocal(rf0[:, :], nf0[:, :])
    rf1 = S(OFC, 1, F32)
    nc.vector.reciprocal(rf1[:, :], nf1[:, :])
    at2Ts = [peT_sb(ex2s[i][:, :], OFC, OFC) for i in range(4)]
    OP2s = []
    for i in range(4):
        o2 = S(32, OFC)                              # row 16 stays 1.0
        nc.gpsimd.memset(o2[:, :], 1.0)
        nc.vector.tensor_copy(o2[0:16, :], mm(16, OFC, vp2[i],
                                              at2Ts[i][:, :])[:, :])
        OP2s.append(o2)
    outs = [to_sb(mm(16, OFC, W(f'wo2T{i}'), OP2s[i][0:17, :]), 16, OFC)
            for i in range(4)]

    # ---- head (sigmoids via exp) ----
    pr0 = S(16, OFC)
    nc.vector.tensor_mul(pr0[:, :], outs[0][:, :], outs[1][:, :])
    pr1 = S(16, OFC)
    nc.vector.tensor_mul(pr1[:, :], outs[3][:, :], outs[2][:, :])
    d0p = mm(OFC, 1, pr0[:, :], W('ones16'))
    d1p = mm(OFC, 1, pr1[:, :], W('ones16'))
    d0n = S(OFC, 1, F32)
    nc.vector.tensor_mul(d0n[:, :], d0p[:, :], rf0[:, :])
    d1n = S(OFC, 1, F32)
    nc.vector.tensor_mul(d1n[:, :], d1p[:, :], rf1[:, :])

    def sigmoid_col(z_in, p, scale, bias, dt):
        """1/(1+exp(-z)) with pre-negated scale/bias arguments."""
        e = S(p, 1, F32)
        nc.scalar.activation(e[:, :], z_in, ACTF.Exp, bias=bias, scale=scale)
        nc.vector.tensor_scalar_add(e[:, :], e[:, :], 1.0)
        r = S(p, 1, F32)
        nc.vector.reciprocal(r[:, :], e[:, :])
        if dt == F32:
            return r
        o = S(p, 1, dt)
        nc.vector.tensor_copy(o[:, :], r[:, :])
        return o

    s0 = sigmoid_col(d0n[:, :], OFC, C('nfcw0'), C('nfcb0'), PE_DT)
    s1 = sigmoid_col(d1n[:, :], OFC, C('nfcw1'), C('nfcb1'), PE_DT)
    hp = P(OFC, 1)
    nc.tensor.matmul(hp[:, :], W('o1aT'), s0[:, :], start=True, stop=False)
    nc.tensor.matmul(hp[:, :], W('o1bT'), s1[:, :], start=False, stop=True)
    hsb = sigmoid_col(hp[:, :], OFC, -1.0, C('no1b'), PE_DT)
    fp = mm(2, 1, W('o2T'), hsb[:, :])
    fin = sigmoid_col(fp[:, :], 2, -1.0, C('no2b'), F32)
    nc.sync.dma_start(y_ap[:, :], fin[0:2, 0:1])


_CACHE = {}


def _build(split=True):
    key = ('nc', split)
    if key in _CACHE:
        return _CACHE[key]
    nc = bass.Bass('TRN2', target_bir_lowering=False, debug=False,
                   num_devices=1)
    wpk_t = nc.dram_tensor('wpk', [128, WPK_F], PE_DT, kind='ExternalInput')
    spk_t = nc.dram_tensor('spk', [128, SPK_F], F32, kind='ExternalInput')
    y = nc.dram_tensor('y', [2, 1], F32, kind='ExternalOutput')
    with tile.TileContext(nc) as tc:
        with ExitStack() as ctx:
            _body(tc, wpk_t, spk_t, y.ap(), ctx)
    if split:
        _slim_tail(nc)
        _split_sync_waits(nc)
    _CACHE[key] = nc
    return nc


def _make_in_map(inputs):
    wpk, spk = _pack_arrays(inputs)
    return {'wpk': wpk, 'spk': spk}


def _install_trace_hook():
    """Shim the missing antenv.axon_hooks module and register the NTFF
    profile hook so run_bass_kernel_spmd(trace=True) works here."""
    import types
    if 'antenv.axon_hooks' not in sys.modules:
        mod = types.ModuleType('antenv.axon_hooks')
        _h = [None]
        mod.set_axon_ntff_profile_hook = lambda h: _h.__setitem__(0, h)
        mod.get_axon_ntff_profile_hook = lambda: _h[0]
        import antenv
        sys.modules['antenv.axon_hooks'] = mod
        antenv.axon_hooks = mod
    from antenv.axon_hooks import (get_axon_ntff_profile_hook,
                                   set_axon_ntff_profile_hook)
    if get_axon_ntff_profile_hook() is None:
        from trn_agent_boot.trn_boot import _ntff_profile_via_ctypes
        set_axon_ntff_profile_hook(
            _ntff_profile_via_ctypes('/opt/axon/libaxon_pjrt.so'))
    import concourse.bass_utils as bu
    bu.upload_artifacts = lambda tmpdir: f"local://{tmpdir}"


def _run(inputs, trace=False, tmpdir=None):
    if trace:
        _install_trace_hook()
    nc = _build()
    in_map = _make_in_map(inputs)
    res = run_bass_kernel_spmd(nc, [in_map] * N_CORES,
                               core_ids=list(range(N_CORES)),
                               trace=trace, tmpdir=tmpdir)
    return res


def kernel(**inputs) -> np.ndarray:
    res = _run(inputs)
    return res.results[0]['y'].reshape(1, 2)


# revision 33
# speedup vs baseline: 1.0047x; 1.0047x over previous
"""Trainium2 Bass kernel for nn_CNN_88098369175791.

Tiny attention/CNN hybrid (batch=1): two time-delay MHAs (E=119) over
sliding wav windows, argmax channel select, LayerNorm, four cross-modal
MHAs (E=16), and an MLP head. The whole model fits on one NeuronCore;
per the sharding hint the program is replicated on all 8 cores (pure
data parallel; with one sample every core computes the same result) and
core 0's output is returned.

Host-side prep does layout only (weight transposes, sliding-window
gathers, bias packing, ones-row augmentation so per-partition biases
ride along inside the matmuls); all arithmetic runs on device with
bf16 PE operands and fp32 PSUM accumulation.

Numerics notes:
- softmax skips the max-subtraction: logits here are provably tiny
  (|l| < 1.5), so exp() is safe and the exp can stream straight out of
  the logits matmul without waiting for a reduction;
- softmax normalization is deferred past the value matmuls and divided
  out where the normalizer lands on a partition axis;
- sigmoids are computed as 1/(1+exp(-z)) so ACT only ever loads the
  Sqrt and Exp tables (a table switch costs ~1.3us).
"""
import itertools
import os
import sys

for _p in ('/opt/trn_rl_repo', '/root/.axon_site/_ro/trn_rl_repo'):
    if os.path.isdir(_p) and _p not in sys.path:
        sys.path.insert(0, _p)

import numpy as np
from contextlib import ExitStack

import concourse.bass as bass
import concourse.tile as tile
from concourse import mybir
from concourse.bass_utils import run_bass_kernel_spmd
from bass_rust import add_dep_helper

F32 = mybir.dt.float32
AX = mybir.AxisListType.X
ALU = mybir.AluOpType
ACTF = mybir.ActivationFunctionType

WL = 140      # window length
TD = 14       # time-delay windows
OFC = 119     # positions / td embed dim
E2 = 16       # cross-modal embed dim
S_TD = float(OFC) ** -0.5
S_CM = float(E2) ** -0.5
N_CORES = 8

PE_MODE = os.environ.get('KPE', 'bf16')
PE_DT = mybir.dt.bfloat16 if PE_MODE == 'bf16' else mybir.dt.float32
PE_NP = np.float32
if PE_MODE == 'bf16':
    import ml_dtypes
    PE_NP = ml_dtypes.bfloat16

INPUT_NAMES = [
    "x", "td_in_w", "td_in_b", "td_out_w", "td_out_b",
    "cm_in_w", "cm_in_b", "cm_out_w", "cm_out_b",
    "mc_w", "mc_b", "max_fc_w", "max_fc_b", "proj_w",
    "ln_g", "ln_b", "fc_w", "fc_b", "out1_w", "out1_b", "out2_w", "out2_b",
]

# ---------------------------------------------------------------------------
# pack layouts (static: computed from shapes only)
# ---------------------------------------------------------------------------


def _mk_layout(specs):
    off = {}
    c = 0
    for name, p, f in specs:
        off[name] = (p, c, f)
        c += f
    return off, c


# PE-operand pack (dtype PE_DT). Order = DMA arrival order; chunk boundaries
# below keep the td-attention front of the kernel fed by the first chunk.
WPK_SPECS = [
    ('winA_aug', 120, TD),        # [wavA windows embed-major ; ones row]
    ('winB_aug', 120, TD),        # adjacent: winAB = joint [120, 28] slice
    ('winGap', 120, 46),          # A @cols 0:14, B @cols 32:46 (vp stacking)
    ('Qpe_aug', 120, 16),         # [eeg_q.T ; ones row]
    ('wqT_aug', 120, OFC),        # [Wq.T ; bq row]
    ('wkT_aug', 120, OFC),        # [Wk.T ; bk row]
    ('ident', 128, 128),
    ('wvT_aug', 120, OFC),        # [Wv.T ; bv row]
    # ---- chunk 1 ends
    ('woB', OFC, 120),            # [Wo | bo col]
    ('eegcm', 16, OFC),
    # ---- chunk 2 ends
    ('winT', TD, 2 * OFC),        # token-major windows [A | B]
    ('mcw0', 16, 1),
    ('mcw1', 16, 1),
    ('mfwT65', 65, 16),           # rows 0:16 = mfwA.T, 32:48 = mfwB.T, 64 = mfb
    ('projcat', 1, 32),
    ('ones16', 16, 1),
    ('stkE', 17, 112),            # [wk2T0 |. wq2T1 |. wq2T2] blocks @0/32/64
    ('stkE2', 17, 16),            # wk2T3 @0
    ('stkA', 17, 48),             # [wq2T0 |. wk2T1] blocks @0/32
    ('stkB', 17, 112),            # [wq2T3 |. .. wk2T2] blocks @0/64
    ('vstkE', 17, 32),            # [wv2T_aug0 | wv2T_aug3]
    ('vstk1', 17, 16),            # wv2T_aug1
    ('vstk2', 17, 16),            # wv2T_aug2
    ('wo2T0', 17, 16), ('wo2T1', 17, 16),
    ('wo2T2', 17, 16), ('wo2T3', 17, 16),
    # ---- chunk 3 ends
    ('o1aT', OFC, OFC),
    ('o1bT', OFC, OFC),
    ('o2T', OFC, 2),
]
WPK_OFF, WPK_F = _mk_layout(WPK_SPECS)
WPK_CHUNK_ENDS = ['wvT_aug', 'eegcm', 'wo2T3', 'o2T']

# f32 side pack: bias columns, DVE scalars, LN input
SPK_SPECS = [
    ('Qf32', OFC, 16),                         # first: tiny DMA, gates LN
    ('no1b', OFC, 1), ('no2b', 2, 1),          # negated (sigmoid-via-exp)
    ('mcb0', 16, 1), ('mcb1', 16, 1),
    ('lng', 16, 1), ('lnb', 16, 1),
    ('nfcw0', OFC, 1), ('nfcw1', OFC, 1),
    ('nfcb0', OFC, 1), ('nfcb1', OFC, 1),
    ('iota16', 1, 16), ('iota14', 1, TD),
]
SPK_OFF, SPK_F = _mk_layout(SPK_SPECS)


def _pack_arrays(inputs):
    """Host-side layout: gathers/transposes/padding only."""
    g = {k: np.asarray(inputs[k], dtype=np.float32) for k in INPUT_NAMES}
    x = g['x'][0, 0]                       # [18,140]
    wavA, eeg, wavB = x[0], x[1:17], x[17]
    eeg_q = eeg[:, WL - OFC:]              # [16,119]
    idx = np.arange(OFC)[:, None] + np.arange(TD)[None, :]
    wA_win = wavA[idx]                     # [119,14]
    wB_win = wavB[idx]

    def aug(m, extra_row):
        return np.concatenate([m, np.asarray(extra_row)[None, :]], axis=0)

    tdw, tdb = g['td_in_w'], g['td_in_b']
    w = {}
    w['winA_aug'] = aug(wA_win, np.ones(TD, np.float32))
    w['winB_aug'] = aug(wB_win, np.ones(TD, np.float32))
    winGap = np.zeros((120, 46), np.float32)
    winGap[:, 0:TD] = w['winA_aug']
    winGap[:, 32:32 + TD] = w['winB_aug']
    w['winGap'] = winGap
    w['Qpe_aug'] = aug(eeg_q.T, np.ones(16, np.float32))
    w['wqT_aug'] = aug(tdw[0:OFC].T, tdb[0:OFC])
    w['wkT_aug'] = aug(tdw[OFC:2 * OFC].T, tdb[OFC:2 * OFC])
    w['wvT_aug'] = aug(tdw[2 * OFC:].T, tdb[2 * OFC:])
    w['ident'] = np.eye(128, dtype=np.float32)
    w['woB'] = np.concatenate([g['td_out_w'], g['td_out_b'][:, None]], axis=1)
    w['eegcm'] = eeg_q
    w['winT'] = np.concatenate([wA_win.T, wB_win.T], axis=1)   # [14,238]
    w['mcw0'] = g['mc_w'][0][:, None]
    w['mcw1'] = g['mc_w'][1][:, None]
    mfwT65 = np.zeros((65, 16), np.float32)
    mfwT65[0:16] = g['max_fc_w'][:, 0:16].T
    mfwT65[32:48] = g['max_fc_w'][:, 16:32].T
    mfwT65[64] = g['max_fc_b']
    w['mfwT65'] = mfwT65
    w['projcat'] = g['proj_w'].reshape(1, 32)
    w['ones16'] = np.ones((16, 1), np.float32)

    cw, cb = g['cm_in_w'], g['cm_in_b']

    def qT(i):   # [17,16] = [Wq2_i.T ; bq2_i]
        return aug(cw[i][0:16].T, cb[i][0:16])

    def kT(i):
        return aug(cw[i][16:32].T, cb[i][16:32])

    def vT(i):
        return aug(cw[i][32:48].T, cb[i][32:48])

    stkE = np.zeros((17, 112), np.float32)
    stkE[:, 0:16] = kT(0)
    stkE[:, 32:48] = qT(1)
    stkE[:, 64:80] = qT(2)
    w['stkE'] = stkE
    w['stkE2'] = kT(3)
    stkA = np.zeros((17, 48), np.float32)
    stkA[:, 0:16] = qT(0)
    stkA[:, 32:48] = kT(1)
    w['stkA'] = stkA
    stkB = np.zeros((17, 112), np.float32)
    stkB[:, 0:16] = qT(3)
    stkB[:, 64:80] = kT(2)
    w['stkB'] = stkB
    w['vstkE'] = np.concatenate([vT(0), vT(3)], axis=1)
    w['vstk1'] = vT(1)
    w['vstk2'] = vT(2)
    for i in range(4):
        w[f'wo2T{i}'] = aug(g['cm_out_w'][i].T, g['cm_out_b'][i])
    w['o1aT'] = g['out1_w'][:, 0:OFC].T
    w['o1bT'] = g['out1_w'][:, OFC:].T
    w['o2T'] = g['out2_w'].T

    wpk = np.zeros((128, WPK_F), dtype=PE_NP)
    for name, (p, c0, f) in WPK_OFF.items():
        wpk[0:p, c0:c0 + f] = w[name].astype(PE_NP)

    s = {}
    s['no1b'] = -g['out1_b'][:, None]
    s['no2b'] = -g['out2_b'][:, None]
    s['mcb0'] = np.full((16, 1), g['mc_b'][0], np.float32)
    s['mcb1'] = np.full((16, 1), g['mc_b'][1], np.float32)
    s['lng'] = g['ln_g'][:, None]
    s['lnb'] = g['ln_b'][:, None]
    s['nfcw0'] = np.full((OFC, 1), -g['fc_w'][0], np.float32)
    s['nfcw1'] = np.full((OFC, 1), -g['fc_w'][1], np.float32)
    s['nfcb0'] = np.full((OFC, 1), -g['fc_b'][0], np.float32)
    s['nfcb1'] = np.full((OFC, 1), -g['fc_b'][1], np.float32)
    s['iota16'] = (np.arange(16, dtype=np.float32) / 1024.0)[None, :]
    s['iota14'] = (np.arange(TD, dtype=np.float32) / 1024.0)[None, :]
    s['Qf32'] = eeg_q.T

    spk = np.zeros((128, SPK_F), dtype=np.float32)
    for name, (p, c0, f) in SPK_OFF.items():
        spk[0:p, c0:c0 + f] = s[name]
    return wpk, spk


# ---------------------------------------------------------------------------
# BIR post-processing: the container's walrus encodes at most one sem-wait
# per instruction; hoist excess waits onto injected NoOp carriers.
# ---------------------------------------------------------------------------


def _split_sync_waits(nc, maxw=1):
    n_new = 0
    for f in nc.m.functions:
        for bb in f.blocks:
            new_insts = []
            for inst in bb.instructions:
                si = inst.sync_info
                if si is not None and si.on_wait and len(si.on_wait) > maxw:
                    waits = list(si.on_wait)
                    keep, extra = waits[:maxw], waits[maxw:]
                    while extra:
                        chunk, extra = extra[:maxw], extra[maxw:]
                        carrier = mybir.InstNoOp(
                            name=f"I-waitsplit-{n_new}",
                            engine=inst.engine,
                            ins=[],
                            outs=[],
                            sync_info=mybir.SyncInfo(on_wait=chunk,
                                                     on_update=[]),
                        )
                        n_new += 1
                        new_insts.append(carrier)
                    si.on_wait = keep
                new_insts.append(inst)
            bb.instructions[:] = new_insts
    return n_new


# ---------------------------------------------------------------------------
# device program
# ---------------------------------------------------------------------------


def _slim_tail(nc):
    """Drop the post-reset all-engine barrier at the end of the tile
    block: every engine halts right after it, the runtime only signals
    completion once all engines halt, and the sem reset it guards has
    already happened under barrier #1."""
    for f in nc.m.functions:
        for bb in f.blocks:
            if not bb.name.endswith('_end'):
                continue
            idx = None
            for i, inst in enumerate(bb.instructions):
                if isinstance(inst, mybir.InstDrain) and getattr(
                        inst, 'is_reset_sema', False):
                    idx = i
            if idx is None:
                continue
            # keep through the reset drain + its ISA payload; drop the
            # trailing barrier (Drain/EventSemaphore pairs)
            keep = bb.instructions[:idx + 1]
            for inst in bb.instructions[idx + 1:]:
                if isinstance(inst, (mybir.InstDrain,
                                     mybir.InstEventSemaphore)):
                    continue
                keep.append(inst)
            bb.instructions[:] = keep


def _body(tc, wpk_t, spk_t, y_ap, ctx):
    nc = tc.nc
    sb = ctx.enter_context(tc.tile_pool(name='sb', bufs=1))
    pp = ctx.enter_context(tc.tile_pool(name='ps', bufs=8, space='PSUM'))
    cnt = itertools.count()

    wpk = sb.tile([128, WPK_F], PE_DT, tag='wpk', name='wpk')
    spk = sb.tile([128, SPK_F], F32, tag='spk', name='spk')
    wap = wpk_t.ap()
    qf_cols = SPK_OFF['Qf32'][2]
    nc.gpsimd.dma_start(spk[:, 0:qf_cols], spk_t.ap()[:, 0:qf_cols])
    c0 = 0
    for k, endname in enumerate(WPK_CHUNK_ENDS):
        p_, cb_, f_ = WPK_OFF[endname]
        c1 = cb_ + f_
        nc.sync.dma_start(wpk[:, c0:c1], wap[:, c0:c1])
        if k == 0:
            nc.sync.dma_start(spk[:, qf_cols:], spk_t.ap()[:, qf_cols:])
        c0 = c1

    def W(name):
        p, c0, f = WPK_OFF[name]
        return wpk[0:p, c0:c0 + f]

    def Wj(name_a, name_b, p):
        pa, ca, fa = WPK_OFF[name_a]
        pb, cb, fb = WPK_OFF[name_b]
        assert ca + fa == cb
        return wpk[0:p, ca:cb + fb]

    def C(name):
        p, c0, f = SPK_OFF[name]
        return spk[0:p, c0:c0 + f]

    def S(p, f, dt=None):
        n = next(cnt)
        return sb.tile([p, f], dt or PE_DT, tag=f's{n}', name=f's{n}')

    def P(p, f, dt=F32):
        return pp.tile([p, f], dt, tag='ps', name=f'ps{next(cnt)}')

    def mm(m, n, lhsT, rhs):
        o = P(m, n)
        nc.tensor.matmul(o[:, :], lhsT, rhs, start=True, stop=True)
        return o

    def to_sb(psum, p, f, dt=None):
        t = S(p, f, dt)
        nc.vector.tensor_copy(t[:, :], psum[:, :])
        return t

    ident = W('ident')

    def peT(in_ap, p, f):
        o = P(f, p, PE_DT)
        nc.tensor.transpose(o[:, :], in_ap, ident[0:p, 0:p])
        return o

    def peT_sb(in_ap, p, f, dt=None):
        return to_sb(peT(in_ap, p, f), f, p, dt)

    INT32 = mybir.dt.int32

    # ---- LayerNorm stats on DVE during the DMA window; the Sqrt is ACT's
    #      first instruction (its table preloads for free), the Exp table
    #      load follows right behind it via a dummy op ----
    Qf = C('Qf32')                                   # [119,16] f32
    ssum = S(OFC, 1, F32)
    nc.vector.reduce_sum(ssum[:, :], Qf, axis=AX)
    sq = S(OFC, 16, F32)
    nc.vector.tensor_mul(sq[:, :], Qf, Qf)
    s2 = S(OFC, 1, F32)
    nc.vector.reduce_sum(s2[:, :], sq[:, :], axis=AX)
    nc.vector.tensor_scalar_mul(s2[:, :], s2[:, :], 1.0 / 16.0)
    mu = S(OFC, 1, F32)
    nc.vector.tensor_scalar_mul(mu[:, :], ssum[:, :], 1.0 / 16.0)
    musq = S(OFC, 1, F32)
    nc.vector.tensor_mul(musq[:, :], mu[:, :], mu[:, :])
    var = S(OFC, 1, F32)
    nc.vector.tensor_sub(var[:, :], s2[:, :], musq[:, :])
    nc.vector.tensor_scalar_add(var[:, :], var[:, :], 1e-5)
    xc = S(OFC, 16, F32)
    nc.vector.tensor_scalar_sub(xc[:, :], Qf, mu[:, 0:1])
    dume = S(1, 1, F32)
    nc.scalar.activation(dume[:, :], nc.const_aps.tensor(0.0, (1, 1)),
                         ACTF.Exp)

    # ---- time-delay attention front (PE) ----
    Qaug = W('Qpe_aug')                              # [120,16]
    winAB = Wj('winA_aug', 'winB_aug', 120)          # [120,28]
    QP = to_sb(mm(OFC, 16, W('wqT_aug'), Qaug), OFC, 16)
    KP = to_sb(mm(OFC, 2 * TD, W('wkT_aug'), winAB), OFC, 2 * TD)
    LG = mm(16, 2 * TD, QP[:, :], KP[:, :])          # [16,28]
    vpG = to_sb(mm(46, OFC, W('winGap'), W('wvT_aug')), 46, OFC)
    u0 = to_sb(mm(OFC, 1, W('eegcm'), W('mcw0')), OFC, 1)
    u1 = to_sb(mm(OFC, 1, W('eegcm'), W('mcw1')), OFC, 1)
    z0 = to_sb(mm(120, 1, W('woB'), u0[:, :]), 120, 1)
    z1 = to_sb(mm(120, 1, W('woB'), u1[:, :]), 120, 1)

    # ---- td softmax (no max-subtraction; normalization deferred) ----
    attn = S(16, 46)                                 # A @0:14, B @32:46
    nc.gpsimd.memset(attn[:, :], 0.0)
    nc.scalar.activation(attn[:, 0:TD], LG[:, 0:TD], ACTF.Exp, scale=S_TD)
    tdexpB = nc.scalar.activation(attn[:, 32:32 + TD], LG[:, TD:2 * TD],
                                  ACTF.Exp, scale=S_TD)
    # LN sqrt AFTER the td exps (its table switch rides the slack before
    # eegln is needed), then a dummy exp to switch the table right back
    # before the cm softmaxes
    std = S(OFC, 1, F32)
    sq_i = nc.scalar.activation(std[:, :], var[:, :], ACTF.Sqrt)
    dume2 = S(1, 1, F32)
    de_i = nc.scalar.activation(dume2[:, :], nc.const_aps.tensor(0.0, (1, 1)),
                                ACTF.Exp)
    add_dep_helper(sq_i.ins, tdexpB.ins, sync=False,
                   reason='ACT table order')
    add_dep_helper(de_i.ins, sq_i.ins, sync=False, reason='ACT table order')
    sums, rsums = [], []
    for h in range(2):
        sm = S(16, 1, F32)
        nc.vector.reduce_sum(sm[:, :], attn[:, 32 * h:32 * h + TD], axis=AX)
        rs = S(16, 1, F32)
        nc.vector.reciprocal(rs[:, :], sm[:, :])
        sums.append(sm)
        rsums.append(rs)
    attnT = peT_sb(attn[:, :], 16, 46)               # [46,16]
    OPs = S(128, 32)                                 # row 119 stays 1.0
    nc.gpsimd.memset(OPs[:, :], 1.0)
    opP = P(OFC, 32)
    nc.tensor.matmul(opP[:, 0:16], vpG[0:TD, :], attnT[0:TD, :],
                     start=True, stop=True)
    nc.tensor.matmul(opP[:, 16:32], vpG[32:32 + TD, :],
                     attnT[32:32 + TD, :], start=True, stop=True)
    nc.vector.tensor_copy(OPs[0:OFC, :], opP[:, :])

    # ---- select_max: vX = OPs_aug.T @ (Wo_aug @ uX) — the z vectors are
    #      precomputed off the critical chain, so the scores follow the
    #      OPs copy directly ----
    cat65 = S(65, 1)
    nc.gpsimd.memset(cat65[:, :], 1.0)
    vAp = mm(16, 1, OPs[0:120, 0:16], z0[:, :])
    vAn = S(16, 1, F32)
    nc.vector.tensor_mul(vAn[:, :], vAp[:, :], rsums[0][:, :])
    nc.vector.tensor_scalar(cat65[0:16, 0:1], vAn[:, :], C('mcb0'), 0.0,
                            op0=ALU.add, op1=ALU.max)
    vBp = mm(16, 1, OPs[0:120, 16:32], z1[:, :])
    vBn = S(16, 1, F32)
    nc.vector.tensor_mul(vBn[:, :], vBp[:, :], rsums[1][:, :])
    nc.vector.tensor_scalar(cat65[32:48, 0:1], vBn[:, :], C('mcb1'), 0.0,
                            op0=ALU.add, op1=ALU.max)
    wtp = mm(1, 16, cat65[:, :], W('mfwT65'))        # [1,16] incl. bias row
    mxw = S(1, 1, F32)
    nc.vector.reduce_max(mxw[:, :], wtp[:, :], axis=AX)
    eq = S(1, 16, F32)
    nc.vector.tensor_scalar(eq[:, :], wtp[:, :], mxw[0:1, 0:1], None,
                            op0=ALU.is_equal)
    msk = S(1, 16, F32)
    nc.vector.tensor_sub(msk[:, :], C('iota16'), eq[:, :])
    mi = S(1, 1, F32)
    nc.vector.tensor_reduce(mi[:, :], msk[:, :], axis=AX, op=ALU.min)
    mic = S(1, 1, F32)
    nc.vector.tensor_scalar(mic[:, :], mi[:, :], 1.0, float(TD - 1) / 1024.0,
                            op0=ALU.add, op1=ALU.min)
    ohr = S(1, TD)
    nc.vector.tensor_scalar(ohr[:, :], C('iota14'), mic[0:1, 0:1], None,
                            op0=ALU.is_equal)
    # (oh.T @ projcat) gives [14,32] = [oh*proj0 | oh*proj1]; multiplying
    # by the token-major windows selects row mi and applies the projection
    # in one step: wX_p = (oh*projX).T @ winT_X
    M1 = to_sb(mm(TD, 32, ohr[:, :], W('projcat')), TD, 32)
    Pp = P(16, 2 * OFC)
    nc.tensor.matmul(Pp[:, 0:OFC], M1[:, 0:16], W('winT')[:, 0:OFC],
                     start=True, stop=True)
    nc.tensor.matmul(Pp[:, OFC:2 * OFC], M1[:, 16:32],
                     W('winT')[:, OFC:2 * OFC], start=True, stop=True)
    PAB = S(32, 2 * OFC)                             # row 16 stays 1.0
    nc.gpsimd.memset(PAB[:, :], 1.0)
    nc.vector.tensor_copy(PAB[0:16, :], Pp[:, :])

    # ---- LayerNorm tail (Newton rsqrt on DVE) + eln-side cm projections;
    #      emitted after the select chain so its DVE/PE work fills gaps
    #      without delaying the critical path (eegln isn't needed until the
    #      cm logits) ----
    rstd = S(OFC, 1, F32)
    nc.vector.reciprocal(rstd[:, :], std[:, :])
    xn = S(OFC, 16)
    nc.vector.tensor_scalar_mul(xn[:, :], xc[:, :], rstd[:, 0:1])
    LNp = peT(xn[:, :], OFC, 16)                     # psum [16,119]
    eegln = S(32, OFC)                               # row 16 stays 1.0
    nc.gpsimd.memset(eegln[:, :], 1.0)
    nc.vector.tensor_scalar(eegln[0:16, :], LNp[:, :], C('lng'), C('lnb'),
                            op0=ALU.mult, op1=ALU.add)
    eln17 = eegln[0:17, :]
    QKe = to_sb(mm(112, OFC, W('stkE'), eln17), 112, OFC)
    KP2_3 = to_sb(mm(16, OFC, W('stkE2'), eln17), 16, OFC)
    vpE = to_sb(mm(OFC, 32, eln17, W('vstkE')), OFC, 32)   # [119, v0|v3]

    # ---- cross-modal attention, 4 heads, stage-major ----
    wA17 = PAB[0:17, 0:OFC]
    wB17 = PAB[0:17, OFC:2 * OFC]
    QKa = to_sb(mm(48, OFC, W('stkA'), wA17), 48, OFC)
    QKb = to_sb(mm(112, OFC, W('stkB'), wB17), 112, OFC)
    vp1 = to_sb(mm(OFC, 16, wA17, W('vstk1')), OFC, 16)
    vp2_ = to_sb(mm(OFC, 16, wB17, W('vstk2')), OFC, 16)
    qp2 = [QKa[0:16, :], QKe[32:48, :], QKe[64:80, :], QKb[0:16, :]]
    kp2 = [QKe[0:16, :], QKa[32:48, :], QKb[64:80, :], KP2_3[:, :]]
    vp2 = [vpE[:, 0:16], vp1[:, :], vp2_[:, :], vpE[:, 16:32]]
    LG2s = [mm(OFC, OFC, qp2[i], kp2[i]) for i in range(4)]
    ex2s, sm2s = [], []
    for i in range(4):
        ex2 = S(OFC, OFC)
        nc.scalar.activation(ex2[:, :], LG2s[i][:, :], ACTF.Exp, scale=S_CM)
        ex2s.append(ex2)
        sm2 = S(OFC, 1, F32)
        nc.vector.reduce_sum(sm2[:, :], ex2[:, :], axis=AX)
        sm2s.append(sm2)
    # normalizer products/reciprocals early (DVE order) so the head tail
    # only waits on the d0p/d1p matmuls
    nf0 = S(OFC, 1, F32)
    nc.vector.tensor_mul(nf0[:, :], sm2s[0][:, :], sm2s[1][:, :])
    nf1 = S(OFC, 1, F32)
    nc.vector.tensor_mul(nf1[:, :], sm2s[3][:, :], sm2s[2][:, :])
    rf0 = S(OFC, 1, F32)
    nc.vector.reciprocal(rf0[:, :], nf0[:, :])
    rf1 = S(OFC, 1, F32)
    nc.vector.reciprocal(rf1[:, :], nf1[:, :])
    at2Ts = [peT_sb(ex2s[i][:, :], OFC, OFC) for i in range(4)]
    OP2s = []
    for i in range(4):
        o2 = S(32, OFC)                              # row 16 stays 1.0
        nc.gpsimd.memset(o2[:, :], 1.0)
        nc.vector.tensor_copy(o2[0:16, :], mm(16, OFC, vp2[i],
                                              at2Ts[i][:, :])[:, :])
        OP2s.append(o2)
    outs = [to_sb(mm(16, OFC, W(f'wo2T{i}'), OP2s[i][0:17, :]), 16, OFC)
            for i in range(4)]

    # ---- head (sigmoids via exp) ----
    pr0 = S(16, OFC)
    nc.vector.tensor_mul(pr0[:, :], outs[0][:, :], outs[1][:, :])
    pr1 = S(16, OFC)
    nc.vector.tensor_mul(pr1[:, :], outs[3][:, :], outs[2][:, :])
    d0p = mm(OFC, 1, pr0[:, :], W('ones16'))
    d1p = mm(OFC, 1, pr1[:, :], W('ones16'))
    d0n = S(OFC, 1, F32)
    nc.vector.tensor_mul(d0n[:, :], d0p[:, :], rf0[:, :])
    d1n = S(OFC, 1, F32)
    nc.vector.tensor_mul(d1n[:, :], d1p[:, :], rf1[:, :])

    def sigmoid_col(z_in, p, scale, bias, dt):
        """1/(1+exp(-z)) with pre-negated scale/bias arguments."""
        e = S(p, 1, F32)
        nc.scalar.activation(e[:, :], z_in, ACTF.Exp, bias=bias, scale=scale)
        nc.vector.tensor_scalar_add(e[:, :], e[:, :], 1.0)
        r = S(p, 1, F32)
        nc.vector.reciprocal(r[:, :], e[:, :])
        if dt == F32:
            return r
        o = S(p, 1, dt)
        nc.vector.tensor_copy(o[:, :], r[:, :])
        return o

    s0 = sigmoid_col(d0n[:, :], OFC, C('nfcw0'), C('nfcb0'), PE_DT)
    s1 = sigmoid_col(d1n[:, :], OFC, C('nfcw1'), C('nfcb1'), PE_DT)
    hp = P(OFC, 1)
    nc.tensor.matmul(hp[:, :], W('o1aT'), s0[:, :], start=True, stop=False)
    nc.tensor.matmul(hp[:, :], W('o1bT'), s1[:, :], start=False, stop=True)
    hsb = sigmoid_col(hp[:, :], OFC, -1.0, C('no1b'), PE_DT)
    fp = mm(2, 1, W('o2T'), hsb[:, :])
    fin = sigmoid_col(fp[:, :], 2, -1.0, C('no2b'), F32)
    nc.sync.dma_start(y_ap[:, :], fin[0:2, 0:1])


_CACHE = {}


def _build(split=True):
    key = ('nc', split)
    if key in _CACHE:
        return _CACHE[key]
    nc = bass.Bass('TRN2', target_bir_lowering=False, debug=False,
                   num_devices=1)
    wpk_t = nc.dram_tensor('wpk', [128, WPK_F], PE_DT, kind='ExternalInput')
    spk_t = nc.dram_tensor('spk', [128, SPK_F], F32, kind='ExternalInput')
    y = nc.dram_tensor('y', [2, 1], F32, kind='ExternalOutput')
    with tile.TileContext(nc) as tc:
        with ExitStack() as ctx:
            _body(tc, wpk_t, spk_t, y.ap(), ctx)
    if split:
        _slim_tail(nc)
        _split_sync_waits(nc)
    _CACHE[key] = nc
    return nc


def _make_in_map(inputs):
    wpk, spk = _pack_arrays(inputs)
    return {'wpk': wpk, 'spk': spk}


def _install_trace_hook():
    """Shim the missing antenv.axon_hooks module and register the NTFF
    profile hook so run_bass_kernel_spmd(trace=True) works here."""
    import types
    if 'antenv.axon_hooks' not in sys.modules:
        mod = types.ModuleType('antenv.axon_hooks')
        _h = [None]
        mod.set_axon_ntff_profile_hook = lambda h: _h.__setitem__(0, h)
        mod.get_axon_ntff_profile_hook = lambda: _h[0]
        import antenv
        sys.modules['antenv.axon_hooks'] = mod
        antenv.axon_hooks = mod
    from antenv.axon_hooks import (get_axon_ntff_profile_hook,
                                   set_axon_ntff_profile_hook)
    if get_axon_ntff_profile_hook() is None:
        from trn_agent_boot.trn_boot import _ntff_profile_via_ctypes
        set_axon_ntff_profile_hook(
            _ntff_profile_via_ctypes('/opt/axon/libaxon_pjrt.so'))
    import concourse.bass_utils as bu
    bu.upload_artifacts = lambda tmpdir: f"local://{tmpdir}"


def _run(inputs, trace=False, tmpdir=None):
    if trace:
        _install_trace_hook()
    nc = _build()
    in_map = _make_in_map(inputs)
    res = run_bass_kernel_spmd(nc, [in_map] * N_CORES,
                               core_ids=list(range(N_CORES)),
                               trace=trace, tmpdir=tmpdir)
    return res


def kernel(**inputs) -> np.ndarray:
    res = _run(inputs)
    return res.results[0]['y'].reshape(1, 2)


# revision 34
# speedup vs baseline: 1.0096x; 1.0049x over previous
"""Trainium2 Bass kernel for nn_CNN_88098369175791.

Tiny attention/CNN hybrid (batch=1): two time-delay MHAs (E=119) over
sliding wav windows, argmax channel select, LayerNorm, four cross-modal
MHAs (E=16), and an MLP head. The whole model fits on one NeuronCore;
per the sharding hint the program is replicated on all 8 cores (pure
data parallel; with one sample every core computes the same result) and
core 0's output is returned.

Host-side prep does layout only (weight transposes, sliding-window
gathers, bias packing, ones-row augmentation so per-partition biases
ride along inside the matmuls); all arithmetic runs on device with
bf16 PE operands and fp32 PSUM accumulation.

Numerics notes:
- softmax skips the max-subtraction: logits here are provably tiny
  (|l| < 1.5), so exp() is safe and the exp can stream straight out of
  the logits matmul without waiting for a reduction;
- softmax normalization is deferred past the value matmuls and divided
  out where the normalizer lands on a partition axis;
- sigmoids are computed as 1/(1+exp(-z)) so ACT only ever loads the
  Sqrt and Exp tables (a table switch costs ~1.3us).
"""
import itertools
import os
import sys

for _p in ('/opt/trn_rl_repo', '/root/.axon_site/_ro/trn_rl_repo'):
    if os.path.isdir(_p) and _p not in sys.path:
        sys.path.insert(0, _p)

import numpy as np
from contextlib import ExitStack

import concourse.bass as bass
import concourse.tile as tile
from concourse import mybir
from concourse.bass_utils import run_bass_kernel_spmd
from bass_rust import add_dep_helper

F32 = mybir.dt.float32
AX = mybir.AxisListType.X
ALU = mybir.AluOpType
ACTF = mybir.ActivationFunctionType

WL = 140      # window length
TD = 14       # time-delay windows
OFC = 119     # positions / td embed dim
E2 = 16       # cross-modal embed dim
S_TD = float(OFC) ** -0.5
S_CM = float(E2) ** -0.5
N_CORES = 8

PE_MODE = os.environ.get('KPE', 'bf16')
PE_DT = mybir.dt.bfloat16 if PE_MODE == 'bf16' else mybir.dt.float32
PE_NP = np.float32
if PE_MODE == 'bf16':
    import ml_dtypes
    PE_NP = ml_dtypes.bfloat16

INPUT_NAMES = [
    "x", "td_in_w", "td_in_b", "td_out_w", "td_out_b",
    "cm_in_w", "cm_in_b", "cm_out_w", "cm_out_b",
    "mc_w", "mc_b", "max_fc_w", "max_fc_b", "proj_w",
    "ln_g", "ln_b", "fc_w", "fc_b", "out1_w", "out1_b", "out2_w", "out2_b",
]

# ---------------------------------------------------------------------------
# pack layouts (static: computed from shapes only)
# ---------------------------------------------------------------------------


def _mk_layout(specs):
    off = {}
    c = 0
    for name, p, f in specs:
        off[name] = (p, c, f)
        c += f
    return off, c


# PE-operand pack (dtype PE_DT). Order = DMA arrival order; chunk boundaries
# below keep the td-attention front of the kernel fed by the first chunk.
WPK_SPECS = [
    ('winA_aug', 120, TD),        # [wavA windows embed-major ; ones row]
    ('winB_aug', 120, TD),        # adjacent: winAB = joint [120, 28] slice
    ('winGap', 120, 46),          # A @cols 0:14, B @cols 32:46 (vp stacking)
    ('Qpe_aug', 120, 16),         # [eeg_q.T ; ones row]
    ('wqT_aug', 120, OFC),        # [Wq.T ; bq row]
    ('wkT_aug', 120, OFC),        # [Wk.T ; bk row]
    ('ident', 128, 128),
    ('wvT_aug', 120, OFC),        # [Wv.T ; bv row]
    # ---- chunk 1 ends
    ('woB', OFC, 120),            # [Wo | bo col]
    ('eegcm', 16, OFC),
    # ---- chunk 2 ends
    ('winT', TD, 2 * OFC),        # token-major windows [A | B]
    ('mcw0', 16, 1),
    ('mcw1', 16, 1),
    ('mfwT65', 65, 16),           # rows 0:16 = mfwA.T, 32:48 = mfwB.T, 64 = mfb
    ('projcat', 1, 32),
    ('ones16', 16, 1),
    ('stkE', 17, 112),            # [wk2T0 |. wq2T1 |. wq2T2] blocks @0/32/64
    ('stkE2', 17, 16),            # wk2T3 @0
    ('stkA', 17, 48),             # [wq2T0 |. wk2T1] blocks @0/32
    ('stkB', 17, 112),            # [wq2T3 |. .. wk2T2] blocks @0/64
    ('vstkE', 17, 32),            # [wv2T_aug0 | wv2T_aug3]
    ('vstk1', 17, 16),            # wv2T_aug1
    ('vstk2', 17, 16),            # wv2T_aug2
    ('wo2T0', 17, 16), ('wo2T1', 17, 16),
    ('wo2T2', 17, 16), ('wo2T3', 17, 16),
    # ---- chunk 3 ends
    ('o1aT', OFC, OFC),
    ('o1bT', OFC, OFC),
    ('o2T', OFC, 2),
]
WPK_OFF, WPK_F = _mk_layout(WPK_SPECS)
WPK_CHUNK_ENDS = ['wvT_aug', 'eegcm', 'wo2T3', 'o2T']

# f32 side pack: bias columns, DVE scalars, LN input
SPK_SPECS = [
    ('Qf32', OFC, 16),                         # first: tiny DMA, gates LN
    ('no1b', OFC, 1), ('no2b', 2, 1),          # negated (sigmoid-via-exp)
    ('mcb0', 16, 1), ('mcb1', 16, 1),
    ('lng', 16, 1), ('lnb', 16, 1),
    ('nfcw0', OFC, 1), ('nfcw1', OFC, 1),
    ('nfcb0', OFC, 1), ('nfcb1', OFC, 1),
    ('iota16', 1, 16), ('iota14', 1, TD),
]
SPK_OFF, SPK_F = _mk_layout(SPK_SPECS)


def _pack_arrays(inputs):
    """Host-side layout: gathers/transposes/padding only."""
    g = {k: np.asarray(inputs[k], dtype=np.float32) for k in INPUT_NAMES}
    x = g['x'][0, 0]                       # [18,140]
    wavA, eeg, wavB = x[0], x[1:17], x[17]
    eeg_q = eeg[:, WL - OFC:]              # [16,119]
    idx = np.arange(OFC)[:, None] + np.arange(TD)[None, :]
    wA_win = wavA[idx]                     # [119,14]
    wB_win = wavB[idx]

    def aug(m, extra_row):
        return np.concatenate([m, np.asarray(extra_row)[None, :]], axis=0)

    tdw, tdb = g['td_in_w'], g['td_in_b']
    w = {}
    w['winA_aug'] = aug(wA_win, np.ones(TD, np.float32))
    w['winB_aug'] = aug(wB_win, np.ones(TD, np.float32))
    winGap = np.zeros((120, 46), np.float32)
    winGap[:, 0:TD] = w['winA_aug']
    winGap[:, 32:32 + TD] = w['winB_aug']
    w['winGap'] = winGap
    w['Qpe_aug'] = aug(eeg_q.T, np.ones(16, np.float32))
    w['wqT_aug'] = aug(tdw[0:OFC].T, tdb[0:OFC])
    w['wkT_aug'] = aug(tdw[OFC:2 * OFC].T, tdb[OFC:2 * OFC])
    w['wvT_aug'] = aug(tdw[2 * OFC:].T, tdb[2 * OFC:])
    w['ident'] = np.eye(128, dtype=np.float32)
    w['woB'] = np.concatenate([g['td_out_w'], g['td_out_b'][:, None]], axis=1)
    w['eegcm'] = eeg_q
    w['winT'] = np.concatenate([wA_win.T, wB_win.T], axis=1)   # [14,238]
    w['mcw0'] = g['mc_w'][0][:, None]
    w['mcw1'] = g['mc_w'][1][:, None]
    mfwT65 = np.zeros((65, 16), np.float32)
    mfwT65[0:16] = g['max_fc_w'][:, 0:16].T
    mfwT65[32:48] = g['max_fc_w'][:, 16:32].T
    mfwT65[64] = g['max_fc_b']
    w['mfwT65'] = mfwT65
    w['projcat'] = g['proj_w'].reshape(1, 32)
    w['ones16'] = np.ones((16, 1), np.float32)

    cw, cb = g['cm_in_w'], g['cm_in_b']

    def qT(i):   # [17,16] = [Wq2_i.T ; bq2_i]
        return aug(cw[i][0:16].T, cb[i][0:16])

    def kT(i):
        return aug(cw[i][16:32].T, cb[i][16:32])

    def vT(i):
        return aug(cw[i][32:48].T, cb[i][32:48])

    stkE = np.zeros((17, 112), np.float32)
    stkE[:, 0:16] = kT(0)
    stkE[:, 32:48] = qT(1)
    stkE[:, 64:80] = qT(2)
    w['stkE'] = stkE
    w['stkE2'] = kT(3)
    stkA = np.zeros((17, 48), np.float32)
    stkA[:, 0:16] = qT(0)
    stkA[:, 32:48] = kT(1)
    w['stkA'] = stkA
    stkB = np.zeros((17, 112), np.float32)
    stkB[:, 0:16] = qT(3)
    stkB[:, 64:80] = kT(2)
    w['stkB'] = stkB
    w['vstkE'] = np.concatenate([vT(0), vT(3)], axis=1)
    w['vstk1'] = vT(1)
    w['vstk2'] = vT(2)
    for i in range(4):
        w[f'wo2T{i}'] = aug(g['cm_out_w'][i].T, g['cm_out_b'][i])
    w['o1aT'] = g['out1_w'][:, 0:OFC].T
    w['o1bT'] = g['out1_w'][:, OFC:].T
    w['o2T'] = g['out2_w'].T

    wpk = np.zeros((128, WPK_F), dtype=PE_NP)
    for name, (p, c0, f) in WPK_OFF.items():
        wpk[0:p, c0:c0 + f] = w[name].astype(PE_NP)

    s = {}
    s['no1b'] = -g['out1_b'][:, None]
    s['no2b'] = -g['out2_b'][:, None]
    s['mcb0'] = np.full((16, 1), g['mc_b'][0], np.float32)
    s['mcb1'] = np.full((16, 1), g['mc_b'][1], np.float32)
    s['lng'] = g['ln_g'][:, None]
    s['lnb'] = g['ln_b'][:, None]
    s['nfcw0'] = np.full((OFC, 1), -g['fc_w'][0], np.float32)
    s['nfcw1'] = np.full((OFC, 1), -g['fc_w'][1], np.float32)
    s['nfcb0'] = np.full((OFC, 1), -g['fc_b'][0], np.float32)
    s['nfcb1'] = np.full((OFC, 1), -g['fc_b'][1], np.float32)
    s['iota16'] = (np.arange(16, dtype=np.float32) / 1024.0)[None, :]
    s['iota14'] = (np.arange(TD, dtype=np.float32) / 1024.0)[None, :]
    s['Qf32'] = eeg_q.T

    spk = np.zeros((128, SPK_F), dtype=np.float32)
    for name, (p, c0, f) in SPK_OFF.items():
        spk[0:p, c0:c0 + f] = s[name]
    return wpk, spk


# ---------------------------------------------------------------------------
# BIR post-processing: the container's walrus encodes at most one sem-wait
# per instruction; hoist excess waits onto injected NoOp carriers.
# ---------------------------------------------------------------------------


def _split_sync_waits(nc, maxw=1):
    n_new = 0
    for f in nc.m.functions:
        for bb in f.blocks:
            new_insts = []
            for inst in bb.instructions:
                si = inst.sync_info
                if si is not None and si.on_wait and len(si.on_wait) > maxw:
                    waits = list(si.on_wait)
                    keep, extra = waits[:maxw], waits[maxw:]
                    while extra:
                        chunk, extra = extra[:maxw], extra[maxw:]
                        carrier = mybir.InstNoOp(
                            name=f"I-waitsplit-{n_new}",
                            engine=inst.engine,
                            ins=[],
                            outs=[],
                            sync_info=mybir.SyncInfo(on_wait=chunk,
                                                     on_update=[]),
                        )
                        n_new += 1
                        new_insts.append(carrier)
                    si.on_wait = keep
                new_insts.append(inst)
            bb.instructions[:] = new_insts
    return n_new


# ---------------------------------------------------------------------------
# device program
# ---------------------------------------------------------------------------


def _slim_tail(nc):
    """Drop the post-reset all-engine barrier at the end of the tile
    block: every engine halts right after it, the runtime only signals
    completion once all engines halt, and the sem reset it guards has
    already happened under barrier #1."""
    for f in nc.m.functions:
        for bb in f.blocks:
            if not bb.name.endswith('_end'):
                continue
            idx = None
            for i, inst in enumerate(bb.instructions):
                if isinstance(inst, mybir.InstDrain) and getattr(
                        inst, 'is_reset_sema', False):
                    idx = i
            if idx is None:
                continue
            # keep through the reset drain + its ISA payload; drop the
            # trailing barrier (Drain/EventSemaphore pairs)
            keep = bb.instructions[:idx + 1]
            for inst in bb.instructions[idx + 1:]:
                if isinstance(inst, (mybir.InstDrain,
                                     mybir.InstEventSemaphore)):
                    continue
                keep.append(inst)
            bb.instructions[:] = keep


def _body(tc, wpk_t, spk_t, y_ap, ctx):
    nc = tc.nc
    sb = ctx.enter_context(tc.tile_pool(name='sb', bufs=1))
    pp = ctx.enter_context(tc.tile_pool(name='ps', bufs=8, space='PSUM'))
    cnt = itertools.count()

    wpk = sb.tile([128, WPK_F], PE_DT, tag='wpk', name='wpk')
    spk = sb.tile([128, SPK_F], F32, tag='spk', name='spk')
    wap = wpk_t.ap()
    qf_cols = SPK_OFF['Qf32'][2]
    nc.gpsimd.dma_start(spk[:, 0:qf_cols], spk_t.ap()[:, 0:qf_cols])
    c0 = 0
    for k, endname in enumerate(WPK_CHUNK_ENDS):
        p_, cb_, f_ = WPK_OFF[endname]
        c1 = cb_ + f_
        nc.sync.dma_start(wpk[:, c0:c1], wap[:, c0:c1])
        if k == 0:
            nc.sync.dma_start(spk[:, qf_cols:], spk_t.ap()[:, qf_cols:])
        c0 = c1

    def W(name):
        p, c0, f = WPK_OFF[name]
        return wpk[0:p, c0:c0 + f]

    def Wj(name_a, name_b, p):
        pa, ca, fa = WPK_OFF[name_a]
        pb, cb, fb = WPK_OFF[name_b]
        assert ca + fa == cb
        return wpk[0:p, ca:cb + fb]

    def C(name):
        p, c0, f = SPK_OFF[name]
        return spk[0:p, c0:c0 + f]

    def S(p, f, dt=None):
        n = next(cnt)
        return sb.tile([p, f], dt or PE_DT, tag=f's{n}', name=f's{n}')

    def P(p, f, dt=F32):
        return pp.tile([p, f], dt, tag='ps', name=f'ps{next(cnt)}')

    def mm(m, n, lhsT, rhs):
        o = P(m, n)
        nc.tensor.matmul(o[:, :], lhsT, rhs, start=True, stop=True)
        return o

    def to_sb(psum, p, f, dt=None):
        t = S(p, f, dt)
        nc.vector.tensor_copy(t[:, :], psum[:, :])
        return t

    ident = W('ident')

    def peT(in_ap, p, f):
        o = P(f, p, PE_DT)
        nc.tensor.transpose(o[:, :], in_ap, ident[0:p, 0:p])
        return o

    def peT_sb(in_ap, p, f, dt=None):
        return to_sb(peT(in_ap, p, f), f, p, dt)

    INT32 = mybir.dt.int32

    # ---- LayerNorm stats on DVE during the DMA window; the Sqrt is ACT's
    #      first instruction (its table preloads for free), the Exp table
    #      load follows right behind it via a dummy op ----
    Qf = C('Qf32')                                   # [119,16] f32
    ssum = S(OFC, 1, F32)
    nc.vector.reduce_sum(ssum[:, :], Qf, axis=AX)
    sq = S(OFC, 16, F32)
    nc.vector.tensor_mul(sq[:, :], Qf, Qf)
    s2 = S(OFC, 1, F32)
    nc.vector.reduce_sum(s2[:, :], sq[:, :], axis=AX)
    nc.vector.tensor_scalar_mul(s2[:, :], s2[:, :], 1.0 / 16.0)
    mu = S(OFC, 1, F32)
    nc.vector.tensor_scalar_mul(mu[:, :], ssum[:, :], 1.0 / 16.0)
    musq = S(OFC, 1, F32)
    nc.vector.tensor_mul(musq[:, :], mu[:, :], mu[:, :])
    var = S(OFC, 1, F32)
    nc.vector.tensor_sub(var[:, :], s2[:, :], musq[:, :])
    nc.vector.tensor_scalar_add(var[:, :], var[:, :], 1e-5)
    xc = S(OFC, 16, F32)
    nc.vector.tensor_scalar_sub(xc[:, :], Qf, mu[:, 0:1])
    dume = S(1, 1, F32)
    nc.scalar.activation(dume[:, :], nc.const_aps.tensor(0.0, (1, 1)),
                         ACTF.Exp)

    # ---- time-delay attention front (PE) ----
    Qaug = W('Qpe_aug')                              # [120,16]
    winAB = Wj('winA_aug', 'winB_aug', 120)          # [120,28]
    QP = to_sb(mm(OFC, 16, W('wqT_aug'), Qaug), OFC, 16)
    KP = to_sb(mm(OFC, 2 * TD, W('wkT_aug'), winAB), OFC, 2 * TD)
    LG = mm(16, 2 * TD, QP[:, :], KP[:, :])          # [16,28]
    vpG = to_sb(mm(46, OFC, W('winGap'), W('wvT_aug')), 46, OFC)
    u0 = to_sb(mm(OFC, 1, W('eegcm'), W('mcw0')), OFC, 1)
    u1 = to_sb(mm(OFC, 1, W('eegcm'), W('mcw1')), OFC, 1)
    z0 = to_sb(mm(120, 1, W('woB'), u0[:, :]), 120, 1)
    z1 = to_sb(mm(120, 1, W('woB'), u1[:, :]), 120, 1)

    # ---- td softmax (no max-subtraction; normalization deferred) ----
    attn = S(16, 46)                                 # A @0:14, B @32:46
    nc.gpsimd.memset(attn[:, :], 0.0)
    nc.scalar.activation(attn[:, 0:TD], LG[:, 0:TD], ACTF.Exp, scale=S_TD)
    tdexpB = nc.scalar.activation(attn[:, 32:32 + TD], LG[:, TD:2 * TD],
                                  ACTF.Exp, scale=S_TD)
    # LN sqrt AFTER the td exps (its table switch rides the slack before
    # eegln is needed), then a dummy exp to switch the table right back
    # before the cm softmaxes
    std = S(OFC, 1, F32)
    sq_i = nc.scalar.activation(std[:, :], var[:, :], ACTF.Sqrt)
    dume2 = S(1, 1, F32)
    de_i = nc.scalar.activation(dume2[:, :], nc.const_aps.tensor(0.0, (1, 1)),
                                ACTF.Exp)
    add_dep_helper(sq_i.ins, tdexpB.ins, sync=False,
                   reason='ACT table order')
    add_dep_helper(de_i.ins, sq_i.ins, sync=False, reason='ACT table order')
    sums, rsums = [], []
    for h in range(2):
        sm = S(16, 1, F32)
        nc.vector.reduce_sum(sm[:, :], attn[:, 32 * h:32 * h + TD], axis=AX)
        rs = S(16, 1, F32)
        nc.vector.reciprocal(rs[:, :], sm[:, :])
        sums.append(sm)
        rsums.append(rs)
    attnT = peT_sb(attn[:, :], 16, 46)               # [46,16]
    OPs = S(128, 32)                                 # row 119 stays 1.0
    nc.gpsimd.memset(OPs[:, :], 1.0)
    opP = P(OFC, 32)
    nc.tensor.matmul(opP[:, 0:16], vpG[0:TD, :], attnT[0:TD, :],
                     start=True, stop=True)
    nc.tensor.matmul(opP[:, 16:32], vpG[32:32 + TD, :],
                     attnT[32:32 + TD, :], start=True, stop=True)
    nc.vector.tensor_copy(OPs[0:OFC, :], opP[:, :])

    # ---- select_max: vX = OPs_aug.T @ (Wo_aug @ uX) — the z vectors are
    #      precomputed off the critical chain, so the scores follow the
    #      OPs copy directly ----
    cat65 = S(65, 1)
    nc.gpsimd.memset(cat65[:, :], 1.0)
    vAp = mm(16, 1, OPs[0:120, 0:16], z0[:, :])
    vAn = S(16, 1, F32)
    nc.vector.tensor_mul(vAn[:, :], vAp[:, :], rsums[0][:, :])
    nc.vector.tensor_scalar(cat65[0:16, 0:1], vAn[:, :], C('mcb0'), 0.0,
                            op0=ALU.add, op1=ALU.max)
    vBp = mm(16, 1, OPs[0:120, 16:32], z1[:, :])
    vBn = S(16, 1, F32)
    nc.vector.tensor_mul(vBn[:, :], vBp[:, :], rsums[1][:, :])
    nc.vector.tensor_scalar(cat65[32:48, 0:1], vBn[:, :], C('mcb1'), 0.0,
                            op0=ALU.add, op1=ALU.max)
    wtp = mm(1, 16, cat65[:, :], W('mfwT65'))        # [1,16] incl. bias row
    mxw = S(1, 1, F32)
    nc.vector.reduce_max(mxw[:, :], wtp[:, :], axis=AX)
    eq = S(1, 16, F32)
    nc.vector.tensor_scalar(eq[:, :], wtp[:, :], mxw[0:1, 0:1], None,
                            op0=ALU.is_equal)
    msk = S(1, 16, F32)
    nc.vector.tensor_sub(msk[:, :], C('iota16'), eq[:, :])
    mi = S(1, 1, F32)
    nc.vector.tensor_reduce(mi[:, :], msk[:, :], axis=AX, op=ALU.min)
    mic = S(1, 1, F32)
    nc.vector.tensor_scalar(mic[:, :], mi[:, :], 1.0, float(TD - 1) / 1024.0,
                            op0=ALU.add, op1=ALU.min)
    ohr = S(1, TD)
    nc.vector.tensor_scalar(ohr[:, :], C('iota14'), mic[0:1, 0:1], None,
                            op0=ALU.is_equal)
    # (oh.T @ projcat) gives [14,32] = [oh*proj0 | oh*proj1]; multiplying
    # by the token-major windows selects row mi and applies the projection
    # in one step: wX_p = (oh*projX).T @ winT_X
    M1 = to_sb(mm(TD, 32, ohr[:, :], W('projcat')), TD, 32)
    Pp = P(16, 2 * OFC)
    nc.tensor.matmul(Pp[:, 0:OFC], M1[:, 0:16], W('winT')[:, 0:OFC],
                     start=True, stop=True)
    nc.tensor.matmul(Pp[:, OFC:2 * OFC], M1[:, 16:32],
                     W('winT')[:, OFC:2 * OFC], start=True, stop=True)
    PAB = S(32, 2 * OFC)                             # row 16 stays 1.0
    nc.gpsimd.memset(PAB[:, :], 1.0)
    nc.vector.tensor_copy(PAB[0:16, 0:OFC], Pp[:, 0:OFC])
    nc.vector.tensor_copy(PAB[0:16, OFC:2 * OFC], Pp[:, OFC:2 * OFC])

    # ---- LayerNorm tail (Newton rsqrt on DVE) + eln-side cm projections;
    #      emitted after the select chain so its DVE/PE work fills gaps
    #      without delaying the critical path (eegln isn't needed until the
    #      cm logits) ----
    rstd = S(OFC, 1, F32)
    nc.vector.reciprocal(rstd[:, :], std[:, :])
    xn = S(OFC, 16)
    nc.vector.tensor_scalar_mul(xn[:, :], xc[:, :], rstd[:, 0:1])
    LNp = peT(xn[:, :], OFC, 16)                     # psum [16,119]
    eegln = S(32, OFC)                               # row 16 stays 1.0
    nc.gpsimd.memset(eegln[:, :], 1.0)
    nc.vector.tensor_scalar(eegln[0:16, :], LNp[:, :], C('lng'), C('lnb'),
                            op0=ALU.mult, op1=ALU.add)
    eln17 = eegln[0:17, :]
    QKe = to_sb(mm(112, OFC, W('stkE'), eln17), 112, OFC)
    KP2_3 = to_sb(mm(16, OFC, W('stkE2'), eln17), 16, OFC)
    vpE = to_sb(mm(OFC, 32, eln17, W('vstkE')), OFC, 32)   # [119, v0|v3]

    # ---- cross-modal attention, 4 heads, stage-major ----
    wA17 = PAB[0:17, 0:OFC]
    wB17 = PAB[0:17, OFC:2 * OFC]
    QKa = to_sb(mm(48, OFC, W('stkA'), wA17), 48, OFC)
    QKb = to_sb(mm(112, OFC, W('stkB'), wB17), 112, OFC)
    vp1 = to_sb(mm(OFC, 16, wA17, W('vstk1')), OFC, 16)
    vp2_ = to_sb(mm(OFC, 16, wB17, W('vstk2')), OFC, 16)
    qp2 = [QKa[0:16, :], QKe[32:48, :], QKe[64:80, :], QKb[0:16, :]]
    kp2 = [QKe[0:16, :], QKa[32:48, :], QKb[64:80, :], KP2_3[:, :]]
    vp2 = [vpE[:, 0:16], vp1[:, :], vp2_[:, :], vpE[:, 16:32]]
    LG2s = [mm(OFC, OFC, qp2[i], kp2[i]) for i in range(4)]
    ex2s, sm2s = [], []
    for i in range(4):
        ex2 = S(OFC, OFC)
        nc.scalar.activation(ex2[:, :], LG2s[i][:, :], ACTF.Exp, scale=S_CM)
        ex2s.append(ex2)
        sm2 = S(OFC, 1, F32)
        nc.vector.reduce_sum(sm2[:, :], ex2[:, :], axis=AX)
        sm2s.append(sm2)
    # normalizer products/reciprocals early (DVE order) so the head tail
    # only waits on the d0p/d1p matmuls
    nf0 = S(OFC, 1, F32)
    nc.vector.tensor_mul(nf0[:, :], sm2s[0][:, :], sm2s[1][:, :])
    nf1 = S(OFC, 1, F32)
    nc.vector.tensor_mul(nf1[:, :], sm2s[3][:, :], sm2s[2][:, :])
    rf0 = S(OFC, 1, F32)
    nc.vector.reciprocal(rf0[:, :], nf0[:, :])
    rf1 = S(OFC, 1, F32)
    nc.vector.reciprocal(rf1[:, :], nf1[:, :])
    at2Ts = [peT_sb(ex2s[i][:, :], OFC, OFC) for i in range(4)]
    OP2s = []
    for i in range(4):
        o2 = S(32, OFC)                              # row 16 stays 1.0
        nc.gpsimd.memset(o2[:, :], 1.0)
        nc.vector.tensor_copy(o2[0:16, :], mm(16, OFC, vp2[i],
                                              at2Ts[i][:, :])[:, :])
        OP2s.append(o2)
    outs = [to_sb(mm(16, OFC, W(f'wo2T{i}'), OP2s[i][0:17, :]), 16, OFC)
            for i in range(4)]

    # ---- head (sigmoids via exp) ----
    pr0 = S(16, OFC)
    nc.vector.tensor_mul(pr0[:, :], outs[0][:, :], outs[1][:, :])
    pr1 = S(16, OFC)
    nc.vector.tensor_mul(pr1[:, :], outs[3][:, :], outs[2][:, :])
    d0p = mm(OFC, 1, pr0[:, :], W('ones16'))
    d1p = mm(OFC, 1, pr1[:, :], W('ones16'))
    d0n = S(OFC, 1, F32)
    nc.vector.tensor_mul(d0n[:, :], d0p[:, :], rf0[:, :])
    d1n = S(OFC, 1, F32)
    nc.vector.tensor_mul(d1n[:, :], d1p[:, :], rf1[:, :])

    def sigmoid_col(z_in, p, scale, bias, dt):
        """1/(1+exp(-z)) with pre-negated scale/bias arguments."""
        e = S(p, 1, F32)
        nc.scalar.activation(e[:, :], z_in, ACTF.Exp, bias=bias, scale=scale)
        nc.vector.tensor_scalar_add(e[:, :], e[:, :], 1.0)
        r = S(p, 1, F32)
        nc.vector.reciprocal(r[:, :], e[:, :])
        if dt == F32:
            return r
        o = S(p, 1, dt)
        nc.vector.tensor_copy(o[:, :], r[:, :])
        return o

    s0 = sigmoid_col(d0n[:, :], OFC, C('nfcw0'), C('nfcb0'), PE_DT)
    s1 = sigmoid_col(d1n[:, :], OFC, C('nfcw1'), C('nfcb1'), PE_DT)
    hp = P(OFC, 1)
    nc.tensor.matmul(hp[:, :], W('o1aT'), s0[:, :], start=True, stop=False)
    nc.tensor.matmul(hp[:, :], W('o1bT'), s1[:, :], start=False, stop=True)
    hsb = sigmoid_col(hp[:, :], OFC, -1.0, C('no1b'), PE_DT)
    fp = mm(2, 1, W('o2T'), hsb[:, :])
    fin = sigmoid_col(fp[:, :], 2, -1.0, C('no2b'), F32)
    nc.sync.dma_start(y_ap[:, :], fin[0:2, 0:1])


_CACHE = {}


def _build(split=True):
    key = ('nc', split)
    if key in _CACHE:
        return _CACHE[key]
    nc = bass.Bass('TRN2', target_bir_lowering=False, debug=False,
                   num_devices=1)
    wpk_t = nc.dram_tensor('wpk', [128, WPK_F], PE_DT, kind='ExternalInput')
    spk_t = nc.dram_tensor('spk', [128, SPK_F], F32, kind='ExternalInput')
    y = nc.dram_tensor('y', [2, 1], F32, kind='ExternalOutput')
    with tile.TileContext(nc) as tc:
        with ExitStack() as ctx:
            _body(tc, wpk_t, spk_t, y.ap(), ctx)
    if split:
        _slim_tail(nc)
        _split_sync_waits(nc)
    _CACHE[key] = nc
    return nc


def _make_in_map(inputs):
    wpk, spk = _pack_arrays(inputs)
    return {'wpk': wpk, 'spk': spk}


def _install_trace_hook():
    """Shim the missing antenv.axon_hooks module and register the NTFF
    profile hook so run_bass_kernel_spmd(trace=True) works here."""
    import types
    if 'antenv.axon_hooks' not in sys.modules:
        mod = types.ModuleType('antenv.axon_hooks')
        _h = [None]
        mod.set_axon_ntff_profile_hook = lambda h: _h.__setitem__(0, h)
        mod.get_axon_ntff_profile_hook = lambda: _h[0]
        import antenv
        sys.modules['antenv.axon_hooks'] = mod
        antenv.axon_hooks = mod
    from antenv.axon_hooks import (get_axon_ntff_profile_hook,
                                   set_axon_ntff_profile_hook)
    if get_axon_ntff_profile_hook() is None:
        from trn_agent_boot.trn_boot import _ntff_profile_via_ctypes
        set_axon_ntff_profile_hook(
            _ntff_profile_via_ctypes('/opt/axon/libaxon_pjrt.so'))
    import concourse.bass_utils as bu
    bu.upload_artifacts = lambda tmpdir: f"local://{tmpdir}"


def _run(inputs, trace=False, tmpdir=None):
    if trace:
        _install_trace_hook()
    nc = _build()
    in_map = _make_in_map(inputs)
    res = run_bass_kernel_spmd(nc, [in_map] * N_CORES,
                               core_ids=list(range(N_CORES)),
                               trace=trace, tmpdir=tmpdir)
    return res


def kernel(**inputs) -> np.ndarray:
    res = _run(inputs)
    return res.results[0]['y'].reshape(1, 2)
